# revision 5
# baseline (speedup 1.0000x reference)
"""AttentiveRouter (MoE routing) Trainium2 Bass kernel.

Full inputs in, full outputs out. Internally: data-parallel over tokens
(B*S = 8192) across 8 NeuronCores, router weights replicated. The final
load-balance reduction over expert_usage is done on host from the
gathered expert_mask (a 64-element reduction).

Per-core pipeline (1024 tokens):
  - PE-transpose x chunks ([128,128] blocks) to get xT (contraction dim
    in partitions)
  - mm1 in "hT form": hT[nb, m] += W1[k, nb-block].T @ xT[k, m-chunk]
    (fp32, W1 block stationary, xT moving, N=512)
  - Gelu + b1 fused into the ACT drain of the mm1 PSUM (bias is
    per-partition in hT layout)
  - mm2: scores[m, e] += hT[nb][:, m-sub].T @ W2[nb]; PSUM initialized
    with a K=1 ones-matmul broadcasting b2
  - epilogue per 128-token tile: softmax (exp with per-partition
    scale=1/T and bias=-rowmax/T, accum_out=Z), top-2 via DVE
    max/max_index, normalized weights, one-hot scatter via iota+is_equal
"""

import numpy as np

import concourse.bass as bass
import concourse.tile as tile
from concourse import bacc, masks, mybir
from concourse.bass_utils import run_bass_kernel_spmd

NCORES = 8
B, S, D, E = 4, 2048, 2048, 64
TOKENS = B * S              # 8192
M_CORE = TOKENS // NCORES   # 1024 tokens per core
CHUNK = 512                 # tokens per mm1 chunk
NCHUNK = M_CORE // CHUNK    # 2
KT = D // 128               # 16 k tiles
NBT = D // 128              # 16 n blocks
TOP_K = 2
CAPACITY = int(1.25 * S)    # 2560 (static, matches reference)

FP = mybir.dt.float32


def build_program():
    nc = bacc.Bacc("TRN2", target_bir_lowering=False, debug=False)

    x_d = nc.dram_tensor("x_shard", [M_CORE, D], FP, kind="ExternalInput").ap()
    w1_d = nc.dram_tensor("W1", [D, D], FP, kind="ExternalInput").ap()
    b1_d = nc.dram_tensor("b1", [D], FP, kind="ExternalInput").ap()
    w2_d = nc.dram_tensor("W2", [D, E], FP, kind="ExternalInput").ap()
    b2_d = nc.dram_tensor("b2", [E], FP, kind="ExternalInput").ap()
    t_d = nc.dram_tensor("temperature", [1], FP, kind="ExternalInput").ap()

    mask_d = nc.dram_tensor("mask_out", [M_CORE, E], FP, kind="ExternalOutput").ap()
    attn_d = nc.dram_tensor("attn_out", [M_CORE, E], FP, kind="ExternalOutput").ap()
    idx_d = nc.dram_tensor(
        "idx_out", [M_CORE, TOP_K], mybir.dt.int32, kind="ExternalOutput"
    ).ap()

    with tile.TileContext(nc) as tc:
        with (
            tc.tile_pool(name="consts", bufs=1) as consts,
            tc.tile_pool(name="w1p", bufs=1) as w1p,
            tc.tile_pool(name="w2p", bufs=1) as w2p,
            tc.tile_pool(name="xstage", bufs=2) as xstage,
            tc.tile_pool(name="xtp", bufs=1) as xtp,
            tc.tile_pool(name="htp", bufs=6) as htp,
            tc.tile_pool(name="epi", bufs=3) as epi,
            tc.tile_pool(name="ps512", bufs=4, space="PSUM") as ps512,
            tc.tile_pool(name="ps64", bufs=4, space="PSUM") as ps64,
        ):
            # ---- constants / preprocessed small tensors ----
            ident = consts.tile([128, 128], FP, tag="ident")
            masks.make_identity(nc, ident[:])

            iota_f = consts.tile([128, E], FP, tag="iota")
            nc.gpsimd.iota(
                iota_f[:], pattern=[[1, E]], base=0, channel_multiplier=0,
                allow_small_or_imprecise_dtypes=True,
            )

            ones1 = consts.tile([1, 128], FP, tag="ones1")
            nc.vector.memset(ones1[:], 1.0)

            b2_sb = consts.tile([1, E], FP, tag="b2sb")
            nc.sync.dma_start(b2_sb[:], b2_d[None, :])

            # b1 rearranged so block nb is a per-partition column: [128, 16]
            b1t = consts.tile([128, NBT], FP, tag="b1t")
            nc.sync.dma_start(b1t[:], b1_d.rearrange("(a b) -> b a", b=128))

            t_sb = consts.tile([1, 1], FP, tag="tsb")
            nc.sync.dma_start(t_sb[:], t_d[None, :])
            invt1 = consts.tile([1, 1], FP, tag="invt1")
            nc.vector.reciprocal(invt1[:], t_sb[:])
            # broadcast 1/T across partitions with a K=1 matmul
            ps_invt = ps512.tile([128, 1], FP, tag="ps512")
            nc.tensor.matmul(ps_invt[:], lhsT=ones1[:], rhs=invt1[:])
            invt_bc = consts.tile([128, 1], FP, tag="invtbc")
            nc.scalar.copy(invt_bc[:], ps_invt[:])
            ninvt_bc = consts.tile([128, 1], FP, tag="ninvtbc")
            nc.scalar.mul(ninvt_bc[:], ps_invt[:], -1.0)

            # ---- resident weights ----
            w1sb = []
            for k in range(KT):
                t = w1p.tile([128, D], FP, tag=f"w1_{k}", name=f"w1_{k}")
                nc.sync.dma_start(t[:], w1_d[k * 128:(k + 1) * 128, :])
                w1sb.append(t)
            w2sb = w2p.tile([128, NBT * E], FP, tag="w2")
            for nb in range(NBT):
                nc.sync.dma_start(
                    w2sb[:, nb * E:(nb + 1) * E], w2_d[nb * 128:(nb + 1) * 128, :]
                )

            xts = [
                xtp.tile([128, CHUNK], FP, tag=f"xt_{k}", name=f"xt_{k}")
                for k in range(KT)
            ]

            for ch in range(NCHUNK):
                m0 = ch * CHUNK
                # ---- stage A: load + transpose x ----
                for ms in range(CHUNK // 128):
                    xs = xstage.tile([128, D], FP, tag="xs")
                    nc.sync.dma_start(xs[:], x_d[m0 + ms * 128:m0 + (ms + 1) * 128, :])
                    for k in range(KT):
                        pt = ps512.tile([128, 128], FP, tag="ps512")
                        nc.tensor.transpose(pt[:], xs[:, k * 128:(k + 1) * 128], ident[:])
                        nc.vector.tensor_copy(
                            xts[k][:, ms * 128:(ms + 1) * 128], pt[:]
                        )

                # scores PSUM tiles for this chunk, pre-loaded with b2
                ps_s = []
                for ms in range(CHUNK // 128):
                    p = ps64.tile([128, E], FP, tag="ps64", name=f"ps_s_{ch}_{ms}")
                    nc.tensor.matmul(p[:], lhsT=ones1[:], rhs=b2_sb[:],
                                     start=True, stop=False)
                    ps_s.append(p)

                # ---- stage B: mm1 (pairs of nb blocks), fused gelu, mm2 ----
                for g in range(NBT // 2):
                    pair = (2 * g, 2 * g + 1)
                    ps_h = {
                        nb: ps512.tile(
                            [128, CHUNK], FP, tag="ps512", name=f"ps_h_{ch}_{nb}"
                        )
                        for nb in pair
                    }
                    for k in range(KT):
                        for nb in pair:
                            nc.tensor.matmul(
                                ps_h[nb][:],
                                lhsT=w1sb[k][:, nb * 128:(nb + 1) * 128],
                                rhs=xts[k][:],
                                start=(k == 0), stop=(k == KT - 1),
                            )
                    for nb in pair:
                        ht = htp.tile([128, CHUNK], FP, tag="ht")
                        nc.scalar.activation(
                            ht[:], ps_h[nb][:],
                            mybir.ActivationFunctionType.Gelu,
                            bias=b1t[:, nb:nb + 1],
                        )
                        for ms in range(CHUNK // 128):
                            nc.tensor.matmul(
                                ps_s[ms][:],
                                lhsT=ht[:, ms * 128:(ms + 1) * 128],
                                rhs=w2sb[:, nb * E:(nb + 1) * E],
                                start=False, stop=(nb == NBT - 1),
                            )

                # ---- stage C: epilogue per 128-token tile ----
                for ms in range(CHUNK // 128):
                    rows = slice(m0 + ms * 128, m0 + (ms + 1) * 128)
                    sc = epi.tile([128, E], FP, tag="sc")
                    nc.vector.tensor_copy(sc[:], ps_s[ms][:])
                    rowmax = epi.tile([128, 1], FP, tag="rowmax")
                    nc.vector.reduce_max(rowmax[:], sc[:], axis=mybir.AxisListType.X)
                    bias_t = epi.tile([128, 1], FP, tag="biast")
                    nc.vector.tensor_tensor(
                        bias_t[:], rowmax[:], ninvt_bc[:], op=mybir.AluOpType.mult
                    )
                    eu = epi.tile([128, E], FP, tag="eu")
                    zsum = epi.tile([128, 1], FP, tag="zsum")
                    nc.scalar.activation(
                        eu[:], sc[:], mybir.ActivationFunctionType.Exp,
                        bias=bias_t[:], scale=invt_bc[:], accum_out=zsum[:],
                    )
                    rz = epi.tile([128, 1], FP, tag="rz")
                    nc.vector.reciprocal(rz[:], zsum[:])
                    probs = epi.tile([128, E], FP, tag="probs")
                    nc.vector.tensor_scalar_mul(probs[:], eu[:], rz[:])
                    nc.sync.dma_start(attn_d[rows, :], probs[:])

                    max8 = epi.tile([128, 8], FP, tag="max8")
                    nc.vector.max(max8[:], probs[:])
                    idx8 = epi.tile([128, 8], mybir.dt.uint32, tag="idx8")
                    nc.vector.max_index(idx8[:], max8[:], probs[:])

                    idx_i = epi.tile([128, TOP_K], mybir.dt.int32, tag="idxi")
                    nc.vector.tensor_copy(idx_i[:], idx8[:, 0:TOP_K])
                    nc.sync.dma_start(idx_d[rows, :], idx_i[:])

                    idx_f = epi.tile([128, TOP_K], FP, tag="idxf")
                    nc.vector.tensor_copy(idx_f[:], idx8[:, 0:TOP_K])

                    s12 = epi.tile([128, 1], FP, tag="s12")
                    nc.vector.tensor_tensor(
                        s12[:], max8[:, 0:1], max8[:, 1:2], op=mybir.AluOpType.add
                    )
                    r12 = epi.tile([128, 1], FP, tag="r12")
                    nc.vector.reciprocal(r12[:], s12[:])
                    wa = epi.tile([128, 1], FP, tag="wa")
                    nc.vector.tensor_tensor(
                        wa[:], max8[:, 0:1], r12[:], op=mybir.AluOpType.mult
                    )
                    wb = epi.tile([128, 1], FP, tag="wb")
                    nc.vector.tensor_tensor(
                        wb[:], max8[:, 1:2], r12[:], op=mybir.AluOpType.mult
                    )

                    t1 = epi.tile([128, E], FP, tag="t1")
                    nc.vector.tensor_scalar(
                        t1[:], iota_f[:], idx_f[:, 0:1], wa[:],
                        op0=mybir.AluOpType.is_equal, op1=mybir.AluOpType.mult,
                    )
                    t2 = epi.tile([128, E], FP, tag="t2")
                    nc.vector.tensor_scalar(
                        t2[:], iota_f[:], idx_f[:, 1:2], wb[:],
                        op0=mybir.AluOpType.is_equal, op1=mybir.AluOpType.mult,
                    )
                    msk = epi.tile([128, E], FP, tag="msk")
                    nc.vector.tensor_tensor(
                        msk[:], t1[:], t2[:], op=mybir.AluOpType.add
                    )
                    nc.sync.dma_start(mask_d[rows, :], msk[:])

    nc.compile()
    return nc


_NC_CACHE = None


def _get_program():
    global _NC_CACHE
    if _NC_CACHE is None:
        _NC_CACHE = build_program()
    return _NC_CACHE


def kernel(x, W1, b1, W2, b2, temperature, _trace=False):
    nc = _get_program()
    xs = np.ascontiguousarray(np.asarray(x, np.float32).reshape(TOKENS, D))
    in_maps = []
    for c in range(NCORES):
        in_maps.append({
            "x_shard": np.ascontiguousarray(xs[c * M_CORE:(c + 1) * M_CORE]),
            "W1": np.asarray(W1, np.float32),
            "b1": np.asarray(b1, np.float32),
            "W2": np.asarray(W2, np.float32),
            "b2": np.asarray(b2, np.float32),
            "temperature": np.asarray(temperature, np.float32),
        })
    kw = {}
    if _trace:
        kw = dict(trace=True)
    res = run_bass_kernel_spmd(nc, in_maps, core_ids=list(range(NCORES)), **kw)
    mask = np.concatenate([res.results[c]["mask_out"] for c in range(NCORES)], axis=0)
    attn = np.concatenate([res.results[c]["attn_out"] for c in range(NCORES)], axis=0)
    idx = np.concatenate([res.results[c]["idx_out"] for c in range(NCORES)], axis=0)

    usage = mask.astype(np.float64).sum(axis=0)           # [E]
    ideal = usage.sum() / E
    lbl = np.mean((usage - ideal) ** 2)
    ecl = np.mean(np.maximum(usage - CAPACITY, 0.0))
    loss = np.float32(lbl + ecl)

    mask = mask.reshape(B, S, E)
    attn = attn.reshape(B, S, E)
    idx = idx.reshape(B, S, TOP_K).astype(np.int32)
    if _trace:
        return (mask, loss, attn, idx), res
    return mask, loss, attn, idx
